# revision 42
# baseline (speedup 1.0000x reference)
"""Multi-head attention (RoPE + pos_bias + mask) Trainium2 Bass kernel.

Sharding: tensor-parallel over heads (2 heads per core, 8 cores), both
batch elements on every core.  Each core computes its heads' attention
and a partial o_proj (its slice of the contraction dim); the host sums
the 8 partials and adds b_o.

Pipeline (v5): the two batch elements are staged so every engine stays
busy and the PE never idles long enough for the HAM clock gate to
re-throttle it to 1.2 GHz:

    proj(b0) -> [ attn(b0) + proj(b1 interleaved) ]
             -> [ attn(b1) + norm(b0) + o_proj(b0) interleaved ]
             -> norm(b1) + o_proj(b1)

"Interleaved" = the filler work is emitted in small units between the
attention kt-groups, so the in-order engine queues dovetail it into the
ACT-bound attention pipeline's gaps.

Other key choices (vs the original 613us baseline):
- All matmuls in fp16: 1 cycle/row guaranteed (f32r drops to 4x below a
  256-wide moving dim), FWL weight loads, and half the DMA bytes.
- The additive (pos_bias + mask) softmax term is factored out as a
  host-precomputed multiplicative expb = exp(pos_bias*scale)*(mask!=0):
  exp runs straight out of PSUM over [128,1024] two-bank tiles (both
  heads per instruction) and one all-fp16 2x-rate DVE multiply applies
  bias+mask.  Masked entries are exact zeros; no -30000 logits.
- Denominators ride as ones-columns inside the V stationary operand,
  are staged to SBUF at partition 64, DMA-respread across partitions,
  and hit ONE batched reciprocal per batch (reciprocal costs FD*8
  cycles regardless of partition count).
"""
import numpy as np

import concourse.bass as bass
import concourse.mybir as mybir
import concourse.tile as tile
from concourse.bass_utils import run_bass_kernel_spmd

B, S, D, H, HD = 2, 2048, 1024, 16, 64
NCORES = 8
T = B * S            # 4096 tokens
KO = D // 128        # 8 contraction subtiles
NCH = T // 512       # 8 projection chunks (4 per batch)
QC = S // 512        # 4 q-chunks per batch

F32 = mybir.dt.float32
F16 = mybir.dt.float16
AF = mybir.ActivationFunctionType

TRACE = False
LAST_RESULT = None   # BassKernelResults of the most recent run (for profiling)

_waitfix_ctr = [0]


def _split_waits(nc, max_waits=1):
    """walrus in this environment accepts only one sync-wait command per
    instruction; TileContext emits several on some (notably the tail
    drain).  Move extras onto single-wait NoOps inserted just before, on
    the same engine queue — identical ordering semantics."""
    total = 0
    for fn in nc.m.functions:
        for bb in fn.blocks:
            out = []
            changed = False
            for ins in bb.instructions:
                si = ins.sync_info
                if si is not None and si.on_wait and len(si.on_wait) > max_waits:
                    waits = list(si.on_wait)
                    for w in waits[:-max_waits]:
                        _waitfix_ctr[0] += 1
                        n = mybir.InstNoOp(
                            name=f"I-waitfix-{_waitfix_ctr[0]}",
                            ins=[], outs=[], engine=ins.engine,
                        )
                        n.sync_info = mybir.SyncInfo(on_wait=[w], on_update=[])
                        out.append(n)
                        total += 1
                    ins.sync_info = mybir.SyncInfo(
                        on_wait=waits[-max_waits:],
                        on_update=list(si.on_update or []),
                    )
                    changed = True
                out.append(ins)
            if changed:
                bb.instructions = out
    return total


def _build():
    nc = bass.Bass()
    xT = nc.declare_dram_parameter("xT", [128, NCH, KO, 512], F16,
                                   isOutput=False)
    wqk = nc.declare_dram_parameter("wqk", [128, KO, 256], F16, isOutput=False)
    wqkb = nc.declare_dram_parameter("wqkb", [2, 256], F16, isOutput=False)
    wv = nc.declare_dram_parameter("wv", [128, KO, 128], F16, isOutput=False)
    wvb = nc.declare_dram_parameter("wvb", [2, 128], F16, isOutput=False)
    wo = nc.declare_dram_parameter("wo", [128, D], F16, isOutput=False)
    cosd = nc.declare_dram_parameter("cosd", [64, S], F16, isOutput=False)
    sind = nc.declare_dram_parameter("sind", [64, S], F16, isOutput=False)
    # expb[qc, kp, kt, 0:512]    = exp(pos_bias)*mask for head0
    # expb[qc, kp, kt, 512:1024] = same for head1  (k = kt*128 + kp)
    expbd = nc.declare_dram_parameter("expb", [QC, 128, 16, 1024], F16,
                                      isOutput=False)
    outp = nc.declare_dram_parameter("out", [T, D], F16, isOutput=True)

    with tile.TileContext(nc) as tc:
        with (
            tc.tile_pool(name="const", bufs=1) as cst,
            tc.tile_pool(name="persist", bufs=1) as pers,
            tc.tile_pool(name="peb", bufs=2) as peb,
            tc.tile_pool(name="pa", bufs=2) as pa,
            tc.tile_pool(name="par", bufs=3) as par,
            tc.tile_pool(name="pex", bufs=3) as pex,
            tc.tile_pool(name="pem", bufs=4) as pem,
            tc.tile_pool(name="pbn", bufs=2) as pbn,
            tc.tile_pool(name="pno", bufs=1) as pno,
            tc.tile_pool(name="pc", bufs=4) as pc,
        ):
            # weights/trig ride the Scalar HWDGE queue, ordered by first
            # use (wo last — o_proj only); the Sync queue carries the x
            # chunks so the first matmul's inputs land within ~6us instead
            # of queueing behind 2MB of constants
            wqk_sb = cst.tile([128, KO, 256], F16)
            nc.scalar.dma_start(wqk_sb[:], wqk[:])
            wv_sb = cst.tile([128, KO, 128], F16)
            nc.scalar.dma_start(wv_sb[:], wv[:])
            cos_sb = cst.tile([128, S], F16)
            sin_sb = cst.tile([128, S], F16)
            for hp in range(2):          # partition halves (head0 / head1)
                nc.scalar.dma_start(cos_sb[64 * hp:64 * hp + 64, :], cosd[:])
                nc.scalar.dma_start(sin_sb[64 * hp:64 * hp + 64, :], sind[:])
            wqkb_sb = cst.tile([2, 256], F16)
            nc.scalar.dma_start(wqkb_sb[:], wqkb[:])
            wvb_sb = cst.tile([2, 128], F16)
            nc.scalar.dma_start(wvb_sb[:], wvb[:])
            wo_sb = cst.tile([128, D], F16)
            nc.scalar.dma_start(wo_sb[:], wo[:])
            ones2 = cst.tile([2, 512], F16)
            nc.vector.memset(ones2[:], 0.0)
            nc.vector.memset(ones2[0:1, :], 1.0)
            ones1x64 = cst.tile([1, 64], F16)
            nc.vector.memset(ones1x64[:], 1.0)

            qTs = [pers.tile([128, S], F16, name=f"qT{b}") for b in range(2)]
            kTs = [pers.tile([128, S], F16, name=f"kT{b}") for b in range(2)]
            # v1[:, g, 0:64] = head0 dims, col 64 = ones, 65:129 = head1
            # dims, col 129 = ones  (g = in-batch token tile of 128)
            v1s = [pers.tile([128, 16, 130], F16, name=f"v1{b}")
                   for b in range(2)]
            vTs = [pers.tile([128, S], F16, name=f"vT{b}") for b in range(2)]
            pvus = [pers.tile([128, S], F16, name=f"pvu{b}")
                    for b in range(2)]
            # denominators staged on partition 64 (aligned with the PV ones
            # row; engines cannot copy across partitions), then DMA-spread
            den = pers.tile([65, 16, 512], F16)
            for b in range(2):
                nc.vector.memset(v1s[b][:, :, 64:65], 1.0)
                nc.vector.memset(v1s[b][:, :, 129:130], 1.0)

            # -------- emission helpers --------

            def proj_chunk_units(b, cc, pqp, pvp_):
                """qkv projection + rope for in-batch 512-token chunk cc of
                batch b, as a list of emission units (closures)."""
                ch = b * 4 + cc
                cs = cc * 512
                state = {}

                def u_dma():
                    xc = pa.tile([128, KO, 512], F16, tag="xc")
                    nc.sync.dma_start(xc[:], xT[:, ch])
                    state["xc"] = xc

                def mk_mm(m):
                    def u_mm():
                        pq = pqp.tile([128, 512], F32, tag="pq")
                        for ko in range(KO):
                            nc.tensor.matmul(
                                pq[:], wqk_sb[:, ko, m * 128:(m + 1) * 128],
                                state["xc"][:, ko], start=(ko == 0),
                                stop=False)
                        nc.tensor.matmul(
                            pq[:], wqkb_sb[:, m * 128:(m + 1) * 128],
                            ones2[:], start=False, stop=True)
                        pq16 = par.tile([128, 512], F16, tag="pq16")
                        nc.scalar.copy(pq16[:], pq[:])
                        state["pq16"] = pq16
                    return u_mm

                def mk_rope(m):
                    def u_rope():
                        pq16 = state["pq16"]
                        t1 = par.tile([128, 512], F16, tag="t1")
                        rot = par.tile([128, 512], F16, tag="rot")
                        nc.vector.tensor_mul(
                            out=t1[:], in0=pq16[:],
                            in1=cos_sb[:, cs:cs + 512])
                        # sin_sb rows are pre-swapped host-side so both SBUF
                        # inputs of each mul share a base partition (walrus
                        # requires it)
                        for hl in range(2):
                            b0 = 64 * hl
                            nc.vector.tensor_mul(
                                out=rot[b0:b0 + 32, :],
                                in0=pq16[b0 + 32:b0 + 64, :],
                                in1=sin_sb[b0 + 32:b0 + 64, cs:cs + 512])
                            nc.vector.tensor_mul(
                                out=rot[b0 + 32:b0 + 64, :],
                                in0=pq16[b0:b0 + 32, :],
                                in1=sin_sb[b0:b0 + 32, cs:cs + 512])
                        dst = qTs[b] if m == 0 else kTs[b]
                        nc.vector.tensor_add(
                            out=dst[:, cs:cs + 512], in0=t1[:], in1=rot[:])
                    return u_rope

                def mk_v(tt):
                    def u_v():
                        g = cc * 4 + tt
                        pv = pvp_.tile([128, 512], F32, tag=pvp_.v_tag)
                        for ko in range(KO):
                            nc.tensor.matmul(
                                pv[:, 0:128],
                                state["xc"][:, ko, tt * 128:(tt + 1) * 128],
                                wv_sb[:, ko], start=(ko == 0), stop=False)
                        nc.tensor.matmul(
                            pv[:, 0:128], ones2[:, 0:128], wvb_sb[:],
                            start=False, stop=True)
                        # strided copy skips the ones column at 64
                        nc.scalar.copy(
                            v1s[b][:, g].rearrange("p (u c) -> p u c",
                                                   u=2)[:, :, 0:64],
                            pv[:, 0:128].rearrange("p (u c) -> p u c", u=2))
                    return u_v

                units = [u_dma, mk_mm(0), mk_rope(0), mk_mm(1), mk_rope(1)]
                units += [mk_v(tt) for tt in range(4)]
                return units

            eb_alive = {}        # qc -> tile; last 2 allocs live
            eb_order = []
            eb_seq = [0]

            def get_eb(qc):
                if qc in eb_alive:
                    return eb_alive[qc]
                t = peb.tile([128, 16, 1024], F16, tag="eb",
                             name=f"eb_{qc}_{eb_seq[0]}")
                eb_seq[0] += 1
                # tiny DVE write creates a WAW dep that pins the DMA start
                # to this point in the DVE stream — without it the idle
                # GPSIMD queue would fire all expb loads at t=0 and starve
                # phase A's critical loads of HBM bandwidth
                nc.vector.memset(t[0:1, 0, 0:1], 0.0)
                # split into 4 sub-DMAs so early kt-groups can start
                # before the whole 4MB chunk lands
                for g in range(4):
                    nc.gpsimd.dma_start(t[:, 4 * g:4 * g + 4],
                                        expbd[qc][:, 4 * g:4 * g + 4])
                eb_order.append(qc)
                eb_alive[qc] = t
                if len(eb_order) > 2:
                    old = eb_order.pop(0)
                    eb_alive.pop(old, None)
                return t

            def attn_batch(b, qc_order, filler, plp, pvp, extra_at=None):
                """attention for batch b; pulls one filler unit per kt.
                extra_at maps a qc-position to a units-factory whose units
                join the filler queue once that q-chunk completes."""
                filler = list(filler)
                pos = [0]

                def fit_next():
                    if pos[0] < len(filler):
                        u = filler[pos[0]]
                        pos[0] += 1
                        return u
                    return None
                for qi, qc in enumerate(qc_order):
                    qs = qc * 512
                    eb = get_eb(qc)
                    if qi + 1 < len(qc_order):
                        get_eb(qc_order[qi + 1])
                    pvt = [pvp.tile([65, 512], F32, tag="pv",
                                    name=f"pv_{b}_{qc}_{hl}")
                           for hl in range(2)]
                    # PV matmuls trail the logits stream by 2 kt-groups so
                    # a wait on the PV accumulator (freed late by the
                    # previous q-chunk's staging copies) never blocks the
                    # QK->exp pipeline in the in-order PE queue
                    pend = []
                    for kt in range(16):
                        ktok = kt * 128
                        pl = plp.tile([128, 1024], F32, tag="pl")
                        for hl in range(2):
                            h0 = 64 * hl
                            nc.tensor.matmul(
                                pl[:, 512 * hl:512 * hl + 512],
                                kTs[b][h0:h0 + 64, ktok:ktok + 128],
                                qTs[b][h0:h0 + 64, qs:qs + 512],
                                start=True, stop=True)
                        ex = pex.tile([128, 1024], F16, tag="ex")
                        nc.scalar.activation(ex[:], pl[:], AF.Exp)
                        exm = pem.tile([128, 1024], F16, tag="exm")
                        nc.vector.tensor_mul(
                            out=exm[:], in0=ex[:], in1=eb[:, kt])
                        pend.append((kt, exm))
                        if len(pend) > 2:
                            fkt, fexm = pend.pop(0)
                            for hl in range(2):
                                nc.tensor.matmul(
                                    pvt[hl][:],
                                    v1s[b][:, fkt, 65 * hl:65 * hl + 65],
                                    fexm[:, 512 * hl:512 * hl + 512],
                                    start=(fkt == 0), stop=(fkt == 15),
                                    skip_group_check=True)
                        u = fit_next()
                        if u is not None:
                            u()
                    for fkt, fexm in pend:
                        for hl in range(2):
                            nc.tensor.matmul(
                                pvt[hl][:],
                                v1s[b][:, fkt, 65 * hl:65 * hl + 65],
                                fexm[:, 512 * hl:512 * hl + 512],
                                start=(fkt == 0), stop=(fkt == 15),
                                skip_group_check=True)
                    for hl in range(2):
                        h0 = 64 * hl
                        idx = b * 8 + qc * 2 + hl
                        nc.vector.tensor_copy(
                            out=den[64:65, idx, :],
                            in_=pvt[hl][64:65, :])
                        nc.vector.tensor_copy(
                            out=pvus[b][h0:h0 + 64, qs:qs + 512],
                            in_=pvt[hl][0:64, :])
                    if extra_at and qi in extra_at:
                        filler.extend(extra_at[qi]())
                while True:
                    u = fit_next()
                    if u is None:
                        break
                    u()

            def norm_units(b, qcs, pqp):
                """deferred softmax normalization for batch b over a
                contiguous subset of q-chunks (their denominators occupy
                contiguous den rows)."""
                state = {}
                i0 = b * 8 + 2 * min(qcs)
                cnt = 2 * len(qcs)

                def u_recip():
                    denp = pno.tile([cnt, 512], F16, tag=f"denp{cnt}")
                    nc.gpsimd.dma_start(denp[:], den[64:65, i0:i0 + cnt])
                    recf = pno.tile([cnt, 512], F32, tag=f"recf{cnt}")
                    nc.vector.reciprocal(recf[:], denp[:])
                    rech = pno.tile([cnt, 512], F16, tag=f"rech{cnt}")
                    nc.vector.tensor_copy(out=rech[:], in_=recf[:])
                    rrow = pno.tile([1, cnt, 512], F16, tag=f"rrow{cnt}")
                    nc.gpsimd.dma_start(rrow[:], rech[:])
                    state["rrow"] = rrow

                def mk_qn(qc):
                    def u_qn():
                        qs = qc * 512
                        bc = pqp.tile([128, 512], F32, tag="pq")
                        bcs = pbn.tile([128, 512], F16, tag="bcs")
                        for hl in range(2):
                            h0 = 64 * hl
                            loc = b * 8 + qc * 2 + hl - i0
                            nc.tensor.matmul(
                                bc[h0:h0 + 64, :], ones1x64[:],
                                state["rrow"][0:1, loc],
                                start=True, stop=True,
                                skip_group_check=True)
                            nc.scalar.copy(bcs[h0:h0 + 64, :],
                                           bc[h0:h0 + 64, :])
                            nc.vector.tensor_mul(
                                out=vTs[b][h0:h0 + 64, qs:qs + 512],
                                in0=pvus[b][h0:h0 + 64, qs:qs + 512],
                                in1=bcs[h0:h0 + 64, :])
                    return u_qn

                return [u_recip] + [mk_qn(qc) for qc in qcs]

            def oproj_units(b, qcs, pqp, copy_engines):
                """partial o_proj for batch b restricted to token tiles of
                the given q-chunks: (matmul, copy, dma) units."""
                units = []
                for qc in qcs:
                    for mt in range(qc * 4, qc * 4 + 4):
                        for n2 in range(2):
                            def u_o(mt=mt, n2=n2):
                                po = pqp.tile([128, 512], F32, tag="pq")
                                nc.tensor.matmul(
                                    po[:],
                                    vTs[b][:, mt * 128:(mt + 1) * 128],
                                    wo_sb[:, n2 * 512:(n2 + 1) * 512],
                                    start=True, stop=True)
                                ob = pc.tile([128, 512], F16, tag="ob")
                                eng = copy_engines[(mt * 2 + n2)
                                                   % len(copy_engines)]
                                if eng == "v":
                                    nc.vector.tensor_copy(out=ob[:],
                                                          in_=po[:])
                                else:
                                    nc.scalar.copy(ob[:], po[:])
                                row = b * S + mt * 128
                                nc.sync.dma_start(
                                    outp[row:row + 128,
                                         n2 * 512:(n2 + 1) * 512], ob[:])
                            units.append(u_o)
                return units

            # -------- phase A(b0): plain, deep-buffered PSUM --------
            with (
                tc.tile_pool(name="pap0", bufs=2, space="PSUM") as pap0,
                tc.tile_pool(name="pav0", bufs=2, space="PSUM") as pav0,
            ):
                pav0.v_tag = "pvv"
                for cc in range(QC):
                    for u in proj_chunk_units(0, cc, pap0, pav0):
                        u()
                    # pre-warm the first two expb chunks mid-phase-A: their
                    # loads start once the DVE reaches this emission point
                    if cc == 1:
                        get_eb(0)
                    elif cc == 2:
                        get_eb(1)

            # -------- phases B/C: shared PSUM budget --------
            with (
                tc.tile_pool(name="plp", bufs=2, space="PSUM") as plp,
                tc.tile_pool(name="pvp", bufs=2, space="PSUM") as pvp,
                tc.tile_pool(name="paux", bufs=2, space="PSUM") as paux,
            ):
                paux.v_tag = "pq"
                # attn(b0) with proj(b1) dovetailed into the kt gaps
                filler_a1 = []
                for cc in range(QC):
                    filler_a1 += proj_chunk_units(1, cc, paux, paux)
                attn_batch(0, [0, 1, 2, 3], filler_a1, plp, pvp)

                # attn(b1) with norm(b0) + o_proj(b0) dovetailed; qc order
                # reversed so the two expb chunks still resident get reused.
                # Once b1's chunks 3+2 finish, their norm + o_proj join the
                # filler stream too — only chunks 1+0 remain for the tail.
                filler_t0 = norm_units(0, [0, 1, 2, 3], paux) + \
                    oproj_units(0, [0, 1, 2, 3], paux, ("v", "v", "s"))

                def mid_b1_units():
                    return (norm_units(1, [2, 3], paux) +
                            oproj_units(1, [2, 3], paux, ("v", "v", "s")))

                attn_batch(1, [3, 2, 1, 0], filler_t0, plp, pvp,
                           extra_at={1: mid_b1_units})

                # tail: norm + o_proj for b1's remaining chunks
                for u in norm_units(1, [0, 1], paux):
                    u()
                for u in oproj_units(1, [0, 1], paux, ("v", "s")):
                    u()

    _split_waits(nc)
    return nc


_nc_cache = None


def _get_nc():
    global _nc_cache
    if _nc_cache is None:
        _nc_cache = _build()
    return _nc_cache


def _prep_inputs(x, pos_bias, sinusoidal_pos, mask, W_qkv, b_qkv, W_o, b_o):
    """Build the 8 per-core input maps (all host-side layout prep)."""
    x = np.asarray(x, np.float32)
    pos_bias = np.asarray(pos_bias, np.float32)
    sp = np.asarray(sinusoidal_pos, np.float32)[0, 0]        # [S, HD]
    mask = np.asarray(mask)
    W_qkv = np.asarray(W_qkv, np.float32)
    b_qkv = np.asarray(b_qkv, np.float32)
    W_o = np.asarray(W_o, np.float32)

    scale = np.float32(1.0 / np.sqrt(HD))

    xflat = x.reshape(T, D)
    # [p, ko, t] -> [p, ch, ko, 512]
    xT_np = np.ascontiguousarray(
        xflat.T.reshape(KO, 128, NCH, 512).transpose(1, 2, 0, 3)
    ).astype(np.float16)

    cos_np = np.cos(sp).T.astype(np.float16)                  # [HD, S]
    sin_t = np.sin(sp).T.astype(np.float32)
    # block-swapped: rows 0:32 hold +sin[32:64] (used for out rows 32:64),
    # rows 32:64 hold -sin[0:32] (used for out rows 0:32)
    sin_np = np.concatenate([sin_t[HD // 2:], -sin_t[:HD // 2]],
                            axis=0).astype(np.float16)

    maskT = (mask[0, 0].T != 0).astype(np.float32)            # [S(k), S(q)]

    # per-head W rows: feature f = h*192 + j (j<64 q, <128 k, <192 v)
    Wh = W_qkv.reshape(H, 3 * HD, D)
    bh = b_qkv.reshape(H, 3 * HD)

    in_maps = []
    for c in range(NCORES):
        h0, h1 = 2 * c, 2 * c + 1
        # q rows scaled by 1/sqrt(HD); k rows unscaled
        Wqk_c = np.concatenate([
            Wh[h0, 0:HD] * scale, Wh[h1, 0:HD] * scale,
            Wh[h0, HD:2 * HD], Wh[h1, HD:2 * HD]], axis=0)    # [256, D]
        bqk_c = np.concatenate([
            bh[h0, 0:HD] * scale, bh[h1, 0:HD] * scale,
            bh[h0, HD:2 * HD], bh[h1, HD:2 * HD]], axis=0)    # [256]
        Wv_c = np.concatenate([Wh[h0, 2 * HD:], Wh[h1, 2 * HD:]], axis=0)
        bv_c = np.concatenate([bh[h0, 2 * HD:], bh[h1, 2 * HD:]], axis=0)

        wqk_np = np.ascontiguousarray(
            Wqk_c.T.reshape(KO, 128, 256).transpose(1, 0, 2)
        ).astype(np.float16)                                   # [128, KO, 256]
        wv_np = np.ascontiguousarray(
            Wv_c.T.reshape(KO, 128, 128).transpose(1, 0, 2)
        ).astype(np.float16)
        wqkb_np = np.zeros((2, 256), np.float16)
        wqkb_np[0] = bqk_c.astype(np.float16)
        wvb_np = np.zeros((2, 128), np.float16)
        wvb_np[0] = bv_c.astype(np.float16)
        wo_np = np.ascontiguousarray(
            W_o[:, 128 * c:128 * (c + 1)].T).astype(np.float16)  # [128, D]

        # expb[qc, kp, kt, hl*512 + qq]
        ebf = np.empty((QC, 128, 16, 1024), np.float16)
        for hl in range(2):
            e = np.exp(pos_bias[0, 2 * c + hl].T * scale) * maskT  # [k, q]
            # [kt, kp, qc, qq] -> [qc, kp, kt, qq]
            ebf[:, :, :, 512 * hl:512 * hl + 512] = (
                e.reshape(16, 128, QC, 512).transpose(2, 1, 0, 3))
        in_maps.append({
            "xT": xT_np, "wqk": wqk_np, "wqkb": wqkb_np,
            "wv": wv_np, "wvb": wvb_np, "wo": wo_np,
            "cosd": cos_np, "sind": sin_np, "expb": ebf,
        })
    return in_maps


def _ensure_profile_hook():
    """Register the axon NTFF profiling hook if the image lacks
    antenv.axon_hooks (needed only for TRACE=True runs)."""
    import sys
    import types
    try:
        from antenv.axon_hooks import get_axon_ntff_profile_hook  # noqa
        return
    except ImportError:
        pass
    try:
        from trn_agent_boot.trn_boot import _ntff_profile_via_ctypes
        hook = _ntff_profile_via_ctypes("/opt/axon/libaxon_pjrt.so")
        mod = types.ModuleType("antenv.axon_hooks")
        mod.get_axon_ntff_profile_hook = lambda: hook
        mod.set_axon_ntff_profile_hook = lambda h: None
        sys.modules["antenv.axon_hooks"] = mod
    except Exception:
        pass


def kernel(x, pos_bias, sinusoidal_pos, mask, W_qkv, b_qkv, W_o, b_o):
    global LAST_RESULT
    if TRACE:
        _ensure_profile_hook()
    in_maps = _prep_inputs(x, pos_bias, sinusoidal_pos, mask,
                           W_qkv, b_qkv, W_o, b_o)
    nc = _get_nc()
    try:
        r = run_bass_kernel_spmd(nc, in_maps, list(range(NCORES)),
                                 trace=TRACE)
    except Exception:
        # occasional transient NRT device errors — retry once
        r = run_bass_kernel_spmd(nc, in_maps, list(range(NCORES)),
                                 trace=TRACE)
    LAST_RESULT = r
    acc = np.zeros((T, D), np.float32)
    for c in range(NCORES):
        acc += r.results[c]["out"].astype(np.float32)
    out = (acc + np.asarray(b_o, np.float32)).astype(np.float32)
    return out.reshape(B, S, D)


# revision 45
# speedup vs baseline: 1.1267x; 1.1267x over previous
"""Multi-head attention (RoPE + pos_bias + mask) Trainium2 Bass kernel.

Sharding: tensor-parallel over heads (2 heads per core, 8 cores), both
batch elements on every core.  Each core computes its heads' attention
and a partial o_proj (its slice of the contraction dim); the host sums
the 8 partials and adds b_o.

Pipeline (v5): the two batch elements are staged so every engine stays
busy and the PE never idles long enough for the HAM clock gate to
re-throttle it to 1.2 GHz:

    proj(b0) -> [ attn(b0) + proj(b1 interleaved) ]
             -> [ attn(b1) + norm(b0) + o_proj(b0) interleaved ]
             -> norm(b1) + o_proj(b1)

"Interleaved" = the filler work is emitted in small units between the
attention kt-groups, so the in-order engine queues dovetail it into the
ACT-bound attention pipeline's gaps.

Other key choices (vs the original 613us baseline):
- All matmuls in fp16: 1 cycle/row guaranteed (f32r drops to 4x below a
  256-wide moving dim), FWL weight loads, and half the DMA bytes.
- The additive (pos_bias + mask) softmax term is factored out as a
  host-precomputed multiplicative expb = exp(pos_bias*scale)*(mask!=0):
  exp runs straight out of PSUM over [128,1024] two-bank tiles (both
  heads per instruction) and one all-fp16 2x-rate DVE multiply applies
  bias+mask.  Masked entries are exact zeros; no -30000 logits.
- Denominators ride as ones-columns inside the V stationary operand,
  are staged to SBUF at partition 64, DMA-respread across partitions,
  and hit ONE batched reciprocal per batch (reciprocal costs FD*8
  cycles regardless of partition count).
"""
import numpy as np

import concourse.bass as bass
import concourse.mybir as mybir
import concourse.tile as tile
from concourse.bass_utils import run_bass_kernel_spmd

B, S, D, H, HD = 2, 2048, 1024, 16, 64
NCORES = 8
T = B * S            # 4096 tokens
KO = D // 128        # 8 contraction subtiles
NCH = T // 512       # 8 projection chunks (4 per batch)
QC = S // 512        # 4 q-chunks per batch

F32 = mybir.dt.float32
F16 = mybir.dt.float16
AF = mybir.ActivationFunctionType

TRACE = False
LAST_RESULT = None   # BassKernelResults of the most recent run (for profiling)

_waitfix_ctr = [0]


def _split_waits(nc, max_waits=1):
    """walrus in this environment accepts only one sync-wait command per
    instruction; TileContext emits several on some (notably the tail
    drain).  Move extras onto single-wait NoOps inserted just before, on
    the same engine queue — identical ordering semantics."""
    total = 0
    for fn in nc.m.functions:
        for bb in fn.blocks:
            out = []
            changed = False
            for ins in bb.instructions:
                si = ins.sync_info
                if si is not None and si.on_wait and len(si.on_wait) > max_waits:
                    waits = list(si.on_wait)
                    for w in waits[:-max_waits]:
                        _waitfix_ctr[0] += 1
                        n = mybir.InstNoOp(
                            name=f"I-waitfix-{_waitfix_ctr[0]}",
                            ins=[], outs=[], engine=ins.engine,
                        )
                        n.sync_info = mybir.SyncInfo(on_wait=[w], on_update=[])
                        out.append(n)
                        total += 1
                    ins.sync_info = mybir.SyncInfo(
                        on_wait=waits[-max_waits:],
                        on_update=list(si.on_update or []),
                    )
                    changed = True
                out.append(ins)
            if changed:
                bb.instructions = out
    return total


def _build():
    nc = bass.Bass()
    xT = nc.declare_dram_parameter("xT", [128, NCH, KO, 512], F16,
                                   isOutput=False)
    wqk = nc.declare_dram_parameter("wqk", [128, KO, 256], F16, isOutput=False)
    wqkb = nc.declare_dram_parameter("wqkb", [2, 256], F16, isOutput=False)
    wv = nc.declare_dram_parameter("wv", [128, KO, 128], F16, isOutput=False)
    wvb = nc.declare_dram_parameter("wvb", [2, 128], F16, isOutput=False)
    wo = nc.declare_dram_parameter("wo", [128, D], F16, isOutput=False)
    cosd = nc.declare_dram_parameter("cosd", [64, S], F16, isOutput=False)
    sind = nc.declare_dram_parameter("sind", [64, S], F16, isOutput=False)
    # expb[qc, kp, kt, 0:512]    = exp(pos_bias)*mask for head0
    # expb[qc, kp, kt, 512:1024] = same for head1  (k = kt*128 + kp)
    expbd = nc.declare_dram_parameter("expb", [QC, 128, 16, 1024], F16,
                                      isOutput=False)
    outp = nc.declare_dram_parameter("out", [T, D], F16, isOutput=True)

    with tile.TileContext(nc) as tc:
        with (
            tc.tile_pool(name="const", bufs=1) as cst,
            tc.tile_pool(name="persist", bufs=1) as pers,
            tc.tile_pool(name="peb", bufs=2) as peb,
            tc.tile_pool(name="pa", bufs=2) as pa,
            tc.tile_pool(name="par", bufs=3) as par,
            tc.tile_pool(name="pex", bufs=3) as pex,
            tc.tile_pool(name="pem", bufs=4) as pem,
            tc.tile_pool(name="pbn", bufs=2) as pbn,
            tc.tile_pool(name="pno", bufs=1) as pno,
            tc.tile_pool(name="pc", bufs=4) as pc,
        ):
            # weights/trig ride the Scalar HWDGE queue, ordered by first
            # use (wo last — o_proj only); the Sync queue carries the x
            # chunks so the first matmul's inputs land within ~6us instead
            # of queueing behind 2MB of constants
            wqk_sb = cst.tile([128, KO, 256], F16)
            nc.scalar.dma_start(wqk_sb[:], wqk[:])
            wv_sb = cst.tile([128, KO, 128], F16)
            nc.scalar.dma_start(wv_sb[:], wv[:])
            cos_sb = cst.tile([128, S], F16)
            sin_sb = cst.tile([128, S], F16)
            for hp in range(2):          # partition halves (head0 / head1)
                nc.scalar.dma_start(cos_sb[64 * hp:64 * hp + 64, :], cosd[:])
                nc.scalar.dma_start(sin_sb[64 * hp:64 * hp + 64, :], sind[:])
            wqkb_sb = cst.tile([2, 256], F16)
            nc.scalar.dma_start(wqkb_sb[:], wqkb[:])
            wvb_sb = cst.tile([2, 128], F16)
            nc.scalar.dma_start(wvb_sb[:], wvb[:])
            wo_sb = cst.tile([128, D], F16)
            nc.scalar.dma_start(wo_sb[:], wo[:])
            ones2 = cst.tile([2, 512], F16)
            nc.vector.memset(ones2[:], 0.0)
            nc.vector.memset(ones2[0:1, :], 1.0)
            ones1x64 = cst.tile([1, 64], F16)
            nc.vector.memset(ones1x64[:], 1.0)

            qTs = [pers.tile([128, S], F16, name=f"qT{b}") for b in range(2)]
            kTs = [pers.tile([128, S], F16, name=f"kT{b}") for b in range(2)]
            # v1[:, g, 0:64] = head0 dims, col 64 = ones, 65:129 = head1
            # dims, col 129 = ones  (g = in-batch token tile of 128)
            v1s = [pers.tile([128, 16, 130], F16, name=f"v1{b}")
                   for b in range(2)]
            vTs = [pers.tile([128, S], F16, name=f"vT{b}") for b in range(2)]
            pvus = [pers.tile([128, S], F16, name=f"pvu{b}")
                    for b in range(2)]
            # denominators staged on partition 64 (aligned with the PV ones
            # row; engines cannot copy across partitions), then DMA-spread
            den = pers.tile([65, 16, 512], F16)
            for b in range(2):
                nc.vector.memset(v1s[b][:, :, 64:65], 1.0)
                nc.vector.memset(v1s[b][:, :, 129:130], 1.0)

            # -------- emission helpers --------

            def proj_chunk_units(b, cc, pqp, pvp_):
                """qkv projection + rope for in-batch 512-token chunk cc of
                batch b, as a list of emission units (closures)."""
                ch = b * 4 + cc
                cs = cc * 512
                state = {}

                def u_dma():
                    xc = pa.tile([128, KO, 512], F16, tag="xc")
                    nc.sync.dma_start(xc[:], xT[:, ch])
                    state["xc"] = xc

                def mk_mm(m):
                    def u_mm():
                        pq = pqp.tile([128, 512], F32, tag="pq")
                        for ko in range(KO):
                            nc.tensor.matmul(
                                pq[:], wqk_sb[:, ko, m * 128:(m + 1) * 128],
                                state["xc"][:, ko], start=(ko == 0),
                                stop=False)
                        nc.tensor.matmul(
                            pq[:], wqkb_sb[:, m * 128:(m + 1) * 128],
                            ones2[:], start=False, stop=True)
                        pq16 = par.tile([128, 512], F16, tag="pq16")
                        nc.scalar.copy(pq16[:], pq[:])
                        state["pq16"] = pq16
                    return u_mm

                def mk_rope(m):
                    def u_rope():
                        pq16 = state["pq16"]
                        t1 = par.tile([128, 512], F16, tag="t1")
                        rot = par.tile([128, 512], F16, tag="rot")
                        nc.vector.tensor_mul(
                            out=t1[:], in0=pq16[:],
                            in1=cos_sb[:, cs:cs + 512])
                        # sin_sb rows are pre-swapped host-side so both SBUF
                        # inputs of each mul share a base partition (walrus
                        # requires it)
                        for hl in range(2):
                            b0 = 64 * hl
                            nc.vector.tensor_mul(
                                out=rot[b0:b0 + 32, :],
                                in0=pq16[b0 + 32:b0 + 64, :],
                                in1=sin_sb[b0 + 32:b0 + 64, cs:cs + 512])
                            nc.vector.tensor_mul(
                                out=rot[b0 + 32:b0 + 64, :],
                                in0=pq16[b0:b0 + 32, :],
                                in1=sin_sb[b0:b0 + 32, cs:cs + 512])
                        dst = qTs[b] if m == 0 else kTs[b]
                        nc.vector.tensor_add(
                            out=dst[:, cs:cs + 512], in0=t1[:], in1=rot[:])
                    return u_rope

                def mk_v(tt):
                    def u_v():
                        g = cc * 4 + tt
                        pv = pvp_.tile([128, 512], F32, tag=pvp_.v_tag)
                        for ko in range(KO):
                            nc.tensor.matmul(
                                pv[:, 0:128],
                                state["xc"][:, ko, tt * 128:(tt + 1) * 128],
                                wv_sb[:, ko], start=(ko == 0), stop=False)
                        nc.tensor.matmul(
                            pv[:, 0:128], ones2[:, 0:128], wvb_sb[:],
                            start=False, stop=True)
                        # strided copy skips the ones column at 64
                        nc.scalar.copy(
                            v1s[b][:, g].rearrange("p (u c) -> p u c",
                                                   u=2)[:, :, 0:64],
                            pv[:, 0:128].rearrange("p (u c) -> p u c", u=2))
                    return u_v

                units = [u_dma, mk_mm(0), mk_rope(0), mk_mm(1), mk_rope(1)]
                units += [mk_v(tt) for tt in range(4)]
                return units

            eb_alive = {}        # qc -> tile; last 2 allocs live
            eb_order = []
            eb_seq = [0]

            def get_eb(qc):
                if qc in eb_alive:
                    return eb_alive[qc]
                t = peb.tile([128, 16, 1024], F16, tag="eb",
                             name=f"eb_{qc}_{eb_seq[0]}")
                eb_seq[0] += 1
                # tiny DVE write creates a WAW dep that pins the DMA start
                # to this point in the DVE stream — without it the idle
                # GPSIMD queue would fire all expb loads at t=0 and starve
                # phase A's critical loads of HBM bandwidth
                nc.vector.memset(t[0:1, 0, 0:1], 0.0)
                # split into 4 sub-DMAs so early kt-groups can start
                # before the whole 4MB chunk lands
                for g in range(4):
                    nc.gpsimd.dma_start(t[:, 4 * g:4 * g + 4],
                                        expbd[qc][:, 4 * g:4 * g + 4])
                eb_order.append(qc)
                eb_alive[qc] = t
                if len(eb_order) > 2:
                    old = eb_order.pop(0)
                    eb_alive.pop(old, None)
                return t

            def attn_batch(b, qc_order, filler, plp, pvp, extra_at=None):
                """attention for batch b; pulls one filler unit per kt.
                extra_at maps a qc-position to a units-factory whose units
                join the filler queue once that q-chunk completes."""
                filler = list(filler)
                pos = [0]

                def fit_next():
                    if pos[0] < len(filler):
                        u = filler[pos[0]]
                        pos[0] += 1
                        return u
                    return None
                for qi, qc in enumerate(qc_order):
                    qs = qc * 512
                    eb = get_eb(qc)
                    if qi + 1 < len(qc_order):
                        get_eb(qc_order[qi + 1])
                    pvt = [pvp.tile([65, 512], F32, tag="pv",
                                    name=f"pv_{b}_{qc}_{hl}")
                           for hl in range(2)]
                    # PV matmuls trail the logits stream by 2 kt-groups so
                    # a wait on the PV accumulator (freed late by the
                    # previous q-chunk's staging copies) never blocks the
                    # QK->exp pipeline in the in-order PE queue
                    pend = []
                    for kt in range(16):
                        ktok = kt * 128
                        pl = plp.tile([128, 1024], F32, tag="pl")
                        for hl in range(2):
                            h0 = 64 * hl
                            nc.tensor.matmul(
                                pl[:, 512 * hl:512 * hl + 512],
                                kTs[b][h0:h0 + 64, ktok:ktok + 128],
                                qTs[b][h0:h0 + 64, qs:qs + 512],
                                start=True, stop=True)
                        ex = pex.tile([128, 1024], F16, tag="ex")
                        nc.scalar.activation(ex[:], pl[:], AF.Exp)
                        exm = pem.tile([128, 1024], F16, tag="exm")
                        nc.vector.tensor_mul(
                            out=exm[:], in0=ex[:], in1=eb[:, kt])
                        pend.append((kt, exm))
                        if len(pend) > 2:
                            fkt, fexm = pend.pop(0)
                            for hl in range(2):
                                nc.tensor.matmul(
                                    pvt[hl][:],
                                    v1s[b][:, fkt, 65 * hl:65 * hl + 65],
                                    fexm[:, 512 * hl:512 * hl + 512],
                                    start=(fkt == 0), stop=(fkt == 15),
                                    skip_group_check=True)
                        u = fit_next()
                        if u is not None:
                            u()
                    for fkt, fexm in pend:
                        for hl in range(2):
                            nc.tensor.matmul(
                                pvt[hl][:],
                                v1s[b][:, fkt, 65 * hl:65 * hl + 65],
                                fexm[:, 512 * hl:512 * hl + 512],
                                start=(fkt == 0), stop=(fkt == 15),
                                skip_group_check=True)
                    for hl in range(2):
                        h0 = 64 * hl
                        idx = b * 8 + qc * 2 + hl
                        nc.vector.tensor_copy(
                            out=den[64:65, idx, :],
                            in_=pvt[hl][64:65, :])
                        nc.vector.tensor_copy(
                            out=pvus[b][h0:h0 + 64, qs:qs + 512],
                            in_=pvt[hl][0:64, :])
                    if extra_at and qi in extra_at:
                        filler.extend(extra_at[qi]())
                while True:
                    u = fit_next()
                    if u is None:
                        break
                    u()

            def norm_units(b, qcs, pqp):
                """deferred softmax normalization for batch b over a
                contiguous subset of q-chunks (their denominators occupy
                contiguous den rows)."""
                state = {}
                i0 = b * 8 + 2 * min(qcs)
                cnt = 2 * len(qcs)

                def u_recip():
                    denp = pno.tile([cnt, 512], F16, tag=f"denp{cnt}")
                    nc.sync.dma_start(denp[:], den[64:65, i0:i0 + cnt])
                    recf = pno.tile([cnt, 512], F32, tag=f"recf{cnt}")
                    nc.vector.reciprocal(recf[:], denp[:])
                    rech = pno.tile([cnt, 512], F16, tag=f"rech{cnt}")
                    nc.vector.tensor_copy(out=rech[:], in_=recf[:])
                    rrow = pno.tile([1, cnt, 512], F16, tag=f"rrow{cnt}")
                    nc.sync.dma_start(rrow[:], rech[:])
                    state["rrow"] = rrow

                def mk_qn(qc):
                    def u_qn():
                        qs = qc * 512
                        bc = pqp.tile([128, 512], F32, tag="pq")
                        bcs = pbn.tile([128, 512], F16, tag="bcs")
                        for hl in range(2):
                            h0 = 64 * hl
                            loc = b * 8 + qc * 2 + hl - i0
                            nc.tensor.matmul(
                                bc[h0:h0 + 64, :], ones1x64[:],
                                state["rrow"][0:1, loc],
                                start=True, stop=True,
                                skip_group_check=True)
                            nc.scalar.copy(bcs[h0:h0 + 64, :],
                                           bc[h0:h0 + 64, :])
                            nc.vector.tensor_mul(
                                out=vTs[b][h0:h0 + 64, qs:qs + 512],
                                in0=pvus[b][h0:h0 + 64, qs:qs + 512],
                                in1=bcs[h0:h0 + 64, :])
                    return u_qn

                return [u_recip] + [mk_qn(qc) for qc in qcs]

            def oproj_units(b, qcs, pqp, copy_engines):
                """partial o_proj for batch b restricted to token tiles of
                the given q-chunks: (matmul, copy, dma) units."""
                units = []
                for qc in qcs:
                    for mt in range(qc * 4, qc * 4 + 4):
                        for n2 in range(2):
                            def u_o(mt=mt, n2=n2):
                                po = pqp.tile([128, 512], F32, tag="pq")
                                nc.tensor.matmul(
                                    po[:],
                                    vTs[b][:, mt * 128:(mt + 1) * 128],
                                    wo_sb[:, n2 * 512:(n2 + 1) * 512],
                                    start=True, stop=True)
                                ob = pc.tile([128, 512], F16, tag="ob")
                                eng = copy_engines[(mt * 2 + n2)
                                                   % len(copy_engines)]
                                if eng == "v":
                                    nc.vector.tensor_copy(out=ob[:],
                                                          in_=po[:])
                                else:
                                    nc.scalar.copy(ob[:], po[:])
                                row = b * S + mt * 128
                                nc.sync.dma_start(
                                    outp[row:row + 128,
                                         n2 * 512:(n2 + 1) * 512], ob[:])
                            units.append(u_o)
                return units

            # -------- phase A(b0): plain, deep-buffered PSUM --------
            with (
                tc.tile_pool(name="pap0", bufs=2, space="PSUM") as pap0,
                tc.tile_pool(name="pav0", bufs=2, space="PSUM") as pav0,
            ):
                pav0.v_tag = "pvv"
                for cc in range(QC):
                    for u in proj_chunk_units(0, cc, pap0, pav0):
                        u()
                    # pre-warm the first two expb chunks mid-phase-A: their
                    # loads start once the DVE reaches this emission point
                    if cc == 1:
                        get_eb(0)
                    elif cc == 2:
                        get_eb(1)

            # -------- phases B/C: shared PSUM budget --------
            with (
                tc.tile_pool(name="plp", bufs=2, space="PSUM") as plp,
                tc.tile_pool(name="pvp", bufs=2, space="PSUM") as pvp,
                tc.tile_pool(name="paux", bufs=2, space="PSUM") as paux,
            ):
                paux.v_tag = "pq"
                # attn(b0) with proj(b1) dovetailed into the kt gaps
                filler_a1 = []
                for cc in range(QC):
                    filler_a1 += proj_chunk_units(1, cc, paux, paux)
                attn_batch(0, [0, 1, 2, 3], filler_a1, plp, pvp)

                # attn(b1) with norm(b0) + o_proj(b0) dovetailed; qc order
                # reversed so the two expb chunks still resident get reused
                filler_t0 = norm_units(0, [0, 1, 2, 3], paux) + \
                    oproj_units(0, [0, 1, 2, 3], paux, ("v", "v", "s"))
                attn_batch(1, [3, 2, 1, 0], filler_t0, plp, pvp)

                # tail: norm(b1) + o_proj(b1)
                for u in norm_units(1, [0, 1, 2, 3], paux):
                    u()
                for u in oproj_units(1, [0, 1, 2, 3], paux, ("v", "s")):
                    u()

    _split_waits(nc)
    return nc


_nc_cache = None


def _get_nc():
    global _nc_cache
    if _nc_cache is None:
        _nc_cache = _build()
    return _nc_cache


def _prep_inputs(x, pos_bias, sinusoidal_pos, mask, W_qkv, b_qkv, W_o, b_o):
    """Build the 8 per-core input maps (all host-side layout prep)."""
    x = np.asarray(x, np.float32)
    pos_bias = np.asarray(pos_bias, np.float32)
    sp = np.asarray(sinusoidal_pos, np.float32)[0, 0]        # [S, HD]
    mask = np.asarray(mask)
    W_qkv = np.asarray(W_qkv, np.float32)
    b_qkv = np.asarray(b_qkv, np.float32)
    W_o = np.asarray(W_o, np.float32)

    scale = np.float32(1.0 / np.sqrt(HD))

    xflat = x.reshape(T, D)
    # [p, ko, t] -> [p, ch, ko, 512]
    xT_np = np.ascontiguousarray(
        xflat.T.reshape(KO, 128, NCH, 512).transpose(1, 2, 0, 3)
    ).astype(np.float16)

    cos_np = np.cos(sp).T.astype(np.float16)                  # [HD, S]
    sin_t = np.sin(sp).T.astype(np.float32)
    # block-swapped: rows 0:32 hold +sin[32:64] (used for out rows 32:64),
    # rows 32:64 hold -sin[0:32] (used for out rows 0:32)
    sin_np = np.concatenate([sin_t[HD // 2:], -sin_t[:HD // 2]],
                            axis=0).astype(np.float16)

    maskT = (mask[0, 0].T != 0).astype(np.float32)            # [S(k), S(q)]

    # per-head W rows: feature f = h*192 + j (j<64 q, <128 k, <192 v)
    Wh = W_qkv.reshape(H, 3 * HD, D)
    bh = b_qkv.reshape(H, 3 * HD)

    in_maps = []
    for c in range(NCORES):
        h0, h1 = 2 * c, 2 * c + 1
        # q rows scaled by 1/sqrt(HD); k rows unscaled
        Wqk_c = np.concatenate([
            Wh[h0, 0:HD] * scale, Wh[h1, 0:HD] * scale,
            Wh[h0, HD:2 * HD], Wh[h1, HD:2 * HD]], axis=0)    # [256, D]
        bqk_c = np.concatenate([
            bh[h0, 0:HD] * scale, bh[h1, 0:HD] * scale,
            bh[h0, HD:2 * HD], bh[h1, HD:2 * HD]], axis=0)    # [256]
        Wv_c = np.concatenate([Wh[h0, 2 * HD:], Wh[h1, 2 * HD:]], axis=0)
        bv_c = np.concatenate([bh[h0, 2 * HD:], bh[h1, 2 * HD:]], axis=0)

        wqk_np = np.ascontiguousarray(
            Wqk_c.T.reshape(KO, 128, 256).transpose(1, 0, 2)
        ).astype(np.float16)                                   # [128, KO, 256]
        wv_np = np.ascontiguousarray(
            Wv_c.T.reshape(KO, 128, 128).transpose(1, 0, 2)
        ).astype(np.float16)
        wqkb_np = np.zeros((2, 256), np.float16)
        wqkb_np[0] = bqk_c.astype(np.float16)
        wvb_np = np.zeros((2, 128), np.float16)
        wvb_np[0] = bv_c.astype(np.float16)
        wo_np = np.ascontiguousarray(
            W_o[:, 128 * c:128 * (c + 1)].T).astype(np.float16)  # [128, D]

        # expb[qc, kp, kt, hl*512 + qq]
        ebf = np.empty((QC, 128, 16, 1024), np.float16)
        for hl in range(2):
            e = np.exp(pos_bias[0, 2 * c + hl].T * scale) * maskT  # [k, q]
            # [kt, kp, qc, qq] -> [qc, kp, kt, qq]
            ebf[:, :, :, 512 * hl:512 * hl + 512] = (
                e.reshape(16, 128, QC, 512).transpose(2, 1, 0, 3))
        in_maps.append({
            "xT": xT_np, "wqk": wqk_np, "wqkb": wqkb_np,
            "wv": wv_np, "wvb": wvb_np, "wo": wo_np,
            "cosd": cos_np, "sind": sin_np, "expb": ebf,
        })
    return in_maps


def _ensure_profile_hook():
    """Register the axon NTFF profiling hook if the image lacks
    antenv.axon_hooks (needed only for TRACE=True runs)."""
    import sys
    import types
    try:
        from antenv.axon_hooks import get_axon_ntff_profile_hook  # noqa
        return
    except ImportError:
        pass
    try:
        from trn_agent_boot.trn_boot import _ntff_profile_via_ctypes
        hook = _ntff_profile_via_ctypes("/opt/axon/libaxon_pjrt.so")
        mod = types.ModuleType("antenv.axon_hooks")
        mod.get_axon_ntff_profile_hook = lambda: hook
        mod.set_axon_ntff_profile_hook = lambda h: None
        sys.modules["antenv.axon_hooks"] = mod
    except Exception:
        pass


def kernel(x, pos_bias, sinusoidal_pos, mask, W_qkv, b_qkv, W_o, b_o):
    global LAST_RESULT
    if TRACE:
        _ensure_profile_hook()
    in_maps = _prep_inputs(x, pos_bias, sinusoidal_pos, mask,
                           W_qkv, b_qkv, W_o, b_o)
    nc = _get_nc()
    try:
        r = run_bass_kernel_spmd(nc, in_maps, list(range(NCORES)),
                                 trace=TRACE)
    except Exception:
        # occasional transient NRT device errors — retry once
        r = run_bass_kernel_spmd(nc, in_maps, list(range(NCORES)),
                                 trace=TRACE)
    LAST_RESULT = r
    acc = np.zeros((T, D), np.float32)
    for c in range(NCORES):
        acc += r.results[c]["out"].astype(np.float32)
    out = (acc + np.asarray(b_o, np.float32)).astype(np.float32)
    return out.reshape(B, S, D)
